# revision 3
# baseline (speedup 1.0000x reference)
"""MultiHeadAttentionWithCache on 8 TRN2 NeuronCores (Bass/Tile).

Sharding: query rows. Cores 0-3 -> batch 0, cores 4-7 -> batch 1. Within a
batch, core j owns query chunks {j, j+4, 11-j, 15-j} (128 rows each) of the
16 chunks -- chosen so causal-mask work is balanced across cores.

Dataflow is fully "transposed" so the PE contracts on partition dims with no
on-device transposes:
  host supplies  hsT [D,512], WqT/WkT/WvT (W.T, [in,out]), WoT (Wo.T [c,j]),
                 kT_cache [H,HD,T] and v_cache [H,T,HD] per batch.
  qT[h]   = (WqT_h).T @ hsT          [HD, 512]   (scale 1/sqrt(HD) folded in)
  k_newT[h], v_new = projections; AllGather over the 4 cores of the batch.
  scoresT = kT_tile.T @ qT           [keys, q]
  eT      = exp(scoresT) (* exp(mask) on partial tiles)   -- no max-subtract:
            |scores| <= ~10 for this problem, fp32 exp is safe.
  avT[h] += V_tile.T @ eT ;  sums += ones.T @ eT
  avT[h] /= sums  (partition_broadcast of reciprocal)
  outT    = WoT.T @ avT_all          [j, q]  -> host reassembles.

All matmuls run in float32r (TF32-like: 11-bit mantissa, full-rate PE).
"""
import math

import numpy as np

B, S, D, H, HD, CACHED = 2, 2048, 2048, 16, 128, 2048
T = CACHED + S
NKT = T // 128          # 32 key tiles
NCH = S // 128          # 16 query chunks per batch
QPC = 512               # query rows per core
NQT = QPC // 128        # 4 chunk slots per core

CHUNK_ASSIGN = [sorted([j, j + 4, 11 - j, 15 - j]) for j in range(4)]
# owner rank / slot for each global chunk
_OWNER = {}
for _j, _chs in enumerate(CHUNK_ASSIGN):
    for _s, _c in enumerate(_chs):
        _OWNER[_c] = (_j, _s)

_PROG_CACHE = {}
TRACE = False
TRACE_KW = {}
LAST_EXEC_NS = None


def _round_f32r(x):
    u = np.ascontiguousarray(x, dtype=np.float32).view(np.uint32).astype(np.uint64)
    lsb = (u >> 12) & 1
    u2 = ((u + 0x7FF + lsb) & np.uint64(0xFFFFF000)).astype(np.uint32)
    return u2.view(np.float32)


def _classify_mask(attention_mask):
    """Block classification from the actual mask data.

    Returns (first, attended, partial_list):
      first[k]    : min slot whose moving range must include key tile k
      attended[k] : key tile computed at all
      partial_list: ordered [(s, k)] blocks needing an exp(mask) multiply
    Merged across all cores and batches (single SPMD program).
    """
    m = np.asarray(attention_mask)  # [B,1,S,T]
    blocks = m.reshape(B, NCH, 128, NKT, 128)
    bmax = blocks.max(axis=(2, 4))  # [B, NCH, NKT]
    bmin = blocks.min(axis=(2, 4))
    full = (np.abs(bmax) < 1e-30) & (np.abs(bmin) < 1e-30)  # mask == 0 block
    zero = bmax <= -200.0                                   # exp -> 0 block

    # slot s of core j holds chunk CHUNK_ASSIGN[j][s]
    att_sk = np.zeros((NQT, NKT), dtype=bool)   # any core/batch attends
    full_sk = np.ones((NQT, NKT), dtype=bool)   # all cores/batches full
    for j in range(4):
        for s in range(NQT):
            c = CHUNK_ASSIGN[j][s]
            att_sk[s] |= (~zero[:, c, :]).any(axis=0)
            full_sk[s] &= full[:, c, :].all(axis=0)

    attended = att_sk.any(axis=0)
    attended[0] = True  # key tile 0 always computed full-width (PSUM start)
    first = np.full(NKT, NQT, dtype=int)
    for k in range(NKT):
        if not attended[k]:
            continue
        s_att = np.where(att_sk[:, k])[0]
        first[k] = s_att.min() if len(s_att) else 0
        first[k] = min(first[k], 2)  # keep moving N >= 256 (f32r full rate)
    first[0] = 0

    partial_list = []
    for k in range(NKT):
        if not attended[k]:
            continue
        for s in range(first[k], NQT):
            if not full_sk[s, k]:
                partial_list.append((s, k))
    return first, attended, partial_list


def _build_program(first, attended, partial_list):
    from concourse import bacc
    import concourse.mybir as mybir
    import concourse.tile as tile

    f32 = mybir.dt.float32
    f32r = mybir.dt.float32r
    nblk = max(len(partial_list), 1)
    blk_idx = {sk: i for i, sk in enumerate(partial_list)}

    nc = bacc.Bacc("TRN2", num_devices=8)
    hsT = nc.dram_tensor("hsT", [D, QPC], f32r, kind="ExternalInput")
    WqT = nc.dram_tensor("WqT", [D, D], f32r, kind="ExternalInput")
    WkT = nc.dram_tensor("WkT", [D, D], f32r, kind="ExternalInput")
    WvT = nc.dram_tensor("WvT", [D, D], f32r, kind="ExternalInput")
    WoT = nc.dram_tensor("WoT", [D, D], f32r, kind="ExternalInput")
    kTc = nc.dram_tensor("kTc", [H, HD, CACHED], f32r, kind="ExternalInput")
    Vc = nc.dram_tensor("Vc", [H, CACHED, HD], f32r, kind="ExternalInput")
    maskexp = nc.dram_tensor("maskexp", [nblk, 128, 128], f32r, kind="ExternalInput")
    outT = nc.dram_tensor("outT", [16, 128, QPC], f32, kind="ExternalOutput")

    KSZ = H * HD * QPC            # k region elems in AG shard
    VSZ = QPC * D
    with tile.TileContext(nc) as tc:
        with tc.tile_pool(name="sb", bufs=1) as sb, \
             tc.tile_pool(name="ps", bufs=1, space="PSUM") as ps, \
             tc.tile_pool(name="dram", bufs=1, space="DRAM") as dram:
            ag_in = dram.tile([KSZ + VSZ], f32r, name="ag_in")
            ag_out = dram.tile([4, KSZ + VSZ], f32r, name="ag_out")

            ones_f = sb.tile([128, 1], f32, name="ones_f")
            nc.gpsimd.memset(ones_f[:], 1.0)
            ones = sb.tile([128, 1], f32r, name="ones")
            nc.vector.tensor_copy(ones[:], ones_f[:])

            hsT_t = sb.tile([128, 16, QPC], f32r, name="hsT_t")
            nc.sync.dma_start(hsT_t[:], hsT.rearrange("(it p) q -> p it q", p=128))

            mb = sb.tile([128, nblk, 128], f32r, name="mb")
            nc.sync.dma_start(mb[:], maskexp.rearrange("n p q -> p n q"))

            qT_all = sb.tile([128, H, QPC], f32r, name="qT_all")
            avT_all = sb.tile([128, H, QPC], f32r, name="avT_all")

            ag_in_k = ag_in[:KSZ].rearrange("(h p q) -> h p q", h=H, p=HD)
            ag_in_v = ag_in[KSZ:].rearrange("(p c) -> p c", p=QPC)
            ag_out_k = ag_out[:, :KSZ].rearrange("r (h p q) -> r h p q", h=H, p=HD)
            ag_out_v = ag_out[:, KSZ:].rearrange("r (p c) -> r p c", p=QPC)

            # ---- K projection: k_newT[h] = WkT_h.T @ hsT  -> ag_in ----
            for h in range(H):
                wk = sb.tile([128, 16, 128], f32r, tag="wqk", bufs=2, name=f"wk{h}")
                nc.sync.dma_start(
                    wk[:], WkT[:, h * 128:(h + 1) * 128]
                    .rearrange("(it p) c -> p it c", p=128))
                pk = ps.tile([128, QPC], f32, tag="A", bufs=2, name=f"pk{h}")
                for it in range(16):
                    nc.tensor.matmul(pk[:], wk[:, it, :], hsT_t[:, it, :],
                                     start=(it == 0), stop=(it == 15))
                kst = sb.tile([128, QPC], f32r, tag="kst", bufs=3, name=f"kst{h}")
                nc.vector.tensor_copy(kst[:], pk[:])
                nc.sync.dma_start(ag_in_k[h], kst[:])

            # ---- V projection: v_new[s*128:,cg*512:] -> ag_in ----
            for cg in range(4):
                pvs = [ps.tile([128, 512], f32, tag=tg, bufs=2, name=f"pv{cg}_{s}")
                       for s, tg in enumerate(("A", "B", "C", "D"))]
                for it in range(16):
                    wv = sb.tile([128, 512], f32r, tag="wv", bufs=3,
                                 name=f"wv{cg}_{it}")
                    nc.sync.dma_start(
                        wv[:], WvT[it * 128:(it + 1) * 128,
                                   cg * 512:(cg + 1) * 512])
                    for s in range(4):
                        nc.tensor.matmul(
                            pvs[s][:], hsT_t[:, it, s * 128:(s + 1) * 128], wv[:],
                            start=(it == 0), stop=(it == 15))
                for s in range(4):
                    vst = sb.tile([128, 512], f32r, tag="vst", bufs=3,
                                  name=f"vst{cg}_{s}")
                    nc.vector.tensor_copy(vst[:], pvs[s][:])
                    nc.sync.dma_start(
                        ag_in_v[s * 128:(s + 1) * 128,
                                cg * 512:(cg + 1) * 512], vst[:])

            # ---- AllGather new K/V within each batch group ----
            nc.gpsimd.collective_compute(
                "AllGather", mybir.AluOpType.bypass,
                replica_groups=[[0, 1, 2, 3], [4, 5, 6, 7]],
                ins=[ag_in.opt()], outs=[ag_out.opt()],
            )

            # ---- Q projection (scale folded into WqT on host) ----
            for h in range(H):
                wq = sb.tile([128, 16, 128], f32r, tag="wqk", bufs=2, name=f"wq{h}")
                nc.sync.dma_start(
                    wq[:], WqT[:, h * 128:(h + 1) * 128]
                    .rearrange("(it p) c -> p it c", p=128))
                pq = ps.tile([128, QPC], f32, tag="B", bufs=2, name=f"pq{h}")
                for it in range(16):
                    nc.tensor.matmul(pq[:], wq[:, it, :], hsT_t[:, it, :],
                                     start=(it == 0), stop=(it == 15))
                nc.vector.tensor_copy(qT_all[:, h, :], pq[:])

            # ---- Attention per head ----
            att_keys = [k for k in range(NKT) if attended[k]]
            last_k = att_keys[-1]
            for h in range(H):
                # key-tile groups of 4 (512 keys per SBUF tile)
                kt_tiles = {}
                v_tiles = {}
                for g in range(8):
                    if not any(attended[k] for k in range(g * 4, g * 4 + 4)):
                        continue
                    kt = sb.tile([128, 512], f32r, tag="kt", bufs=4,
                                 name=f"kt{h}_{g}")
                    vt = sb.tile([128, 4, HD], f32r, tag="vt", bufs=4,
                                 name=f"vt{h}_{g}")
                    if g < 4:  # cached
                        nc.sync.dma_start(
                            kt[:], kTc[h, :, g * 512:(g + 1) * 512])
                        nc.sync.dma_start(
                            vt[:], Vc[h, g * 512:(g + 1) * 512, :]
                            .rearrange("(n p) d -> p n d", p=128))
                    else:
                        for i in range(4):
                            c = (g - 4) * 4 + i  # new chunk index
                            r, sl = _OWNER[c]
                            nc.sync.dma_start(
                                kt[:, i * 128:(i + 1) * 128],
                                ag_out_k[r, h, :, sl * 128:(sl + 1) * 128])
                            nc.sync.dma_start(
                                vt[:, i, :],
                                ag_out_v[r, sl * 128:(sl + 1) * 128,
                                         h * 128:(h + 1) * 128])
                    kt_tiles[g] = kt
                    v_tiles[g] = vt

                avp = ps.tile([128, QPC], f32, tag="C", bufs=2, name=f"av{h}")
                smp = ps.tile([1, QPC], f32, tag="D", bufs=2, name=f"sm{h}")
                for k in att_keys:
                    g, i = k // 4, k % 4
                    f = first[k]
                    n = QPC - f * 128
                    sc = ps.tile([128, n], f32, tag="A", bufs=2, name=f"sc{h}_{k}")
                    nc.tensor.matmul(
                        sc[:], kt_tiles[g][:, i * 128:(i + 1) * 128],
                        qT_all[:, h, f * 128:], start=True, stop=True)
                    eT = sb.tile([128, n], f32r, tag="eT", bufs=4,
                                 name=f"eT{h}_{k}")
                    nc.scalar.activation(eT[:], sc[:],
                                         mybir.ActivationFunctionType.Exp)
                    for s in range(f, NQT):
                        bi = blk_idx.get((s, k))
                        if bi is not None:
                            off = (s - f) * 128
                            nc.vector.tensor_mul(
                                eT[:, off:off + 128], eT[:, off:off + 128],
                                mb[:, bi, :])
                    nc.tensor.matmul(avp[:, f * 128:], v_tiles[g][:, i, :], eT[:],
                                     start=(k == 0), stop=(k == last_k))
                    nc.tensor.matmul(smp[:, f * 128:], ones[:], eT[:],
                                     start=(k == 0), stop=(k == last_k))

                recip = sb.tile([1, QPC], f32, tag="recip", bufs=2,
                                name=f"rc{h}")
                nc.vector.reciprocal(recip[:], smp[:])
                rep = sb.tile([128, QPC], f32, tag="rep", bufs=2, name=f"rep{h}")
                nc.gpsimd.partition_broadcast(rep[:], recip[:])
                nc.vector.tensor_mul(avT_all[:, h, :], avp[:], rep[:])

            # ---- Output projection: outT[jt] = sum_ct WoT_t.T @ avT ----
            for jt in range(16):
                wo = sb.tile([128, 16, 128], f32r, tag="wo", bufs=2,
                             name=f"wo{jt}")
                nc.sync.dma_start(
                    wo[:], WoT[:, jt * 128:(jt + 1) * 128]
                    .rearrange("(ct p) c -> p ct c", p=128))
                po = ps.tile([128, QPC], f32, tag="B", bufs=2, name=f"po{jt}")
                for ct in range(16):
                    nc.tensor.matmul(po[:], wo[:, ct, :], avT_all[:, ct, :],
                                     start=(ct == 0), stop=(ct == 15))
                ob = sb.tile([128, QPC], f32, tag="ob", bufs=2, name=f"ob{jt}")
                nc.vector.tensor_copy(ob[:], po[:])
                nc.sync.dma_start(outT[jt], ob[:])

    nc.compile()
    return nc


def kernel(hidden_states, Wq, Wk, Wv, Wo, k_cache, v_cache, attention_mask):
    global LAST_EXEC_NS
    from concourse import bass_utils

    hs = np.asarray(hidden_states, dtype=np.float32)
    mask = np.asarray(attention_mask, dtype=np.float32)

    first, attended, partial_list = _classify_mask(mask)
    sig = (tuple(first), tuple(bool(a) for a in attended), tuple(partial_list))
    if sig not in _PROG_CACHE:
        _PROG_CACHE[sig] = _build_program(first, attended, partial_list)
    nc = _PROG_CACHE[sig]

    WqTs = _round_f32r(np.asarray(Wq, np.float32).T * (HD ** -0.5))
    WkTs = _round_f32r(np.asarray(Wk, np.float32).T)
    WvTs = _round_f32r(np.asarray(Wv, np.float32).T)
    WoTs = _round_f32r(np.asarray(Wo, np.float32).T)
    kTc_b = [_round_f32r(np.ascontiguousarray(
        np.asarray(k_cache, np.float32)[b].transpose(0, 2, 1))) for b in range(B)]
    Vc_b = [_round_f32r(np.asarray(v_cache, np.float32)[b]) for b in range(B)]

    nblk = max(len(partial_list), 1)
    in_maps = []
    for ci in range(8):
        b, j = ci // 4, ci % 4
        chunks = CHUNK_ASSIGN[j]
        hsT_c = np.concatenate(
            [hs[b, c * 128:(c + 1) * 128, :] for c in chunks], axis=0).T
        me = np.zeros((nblk, 128, 128), np.float32)
        for i, (s, k) in enumerate(partial_list):
            c = chunks[s]
            blk = mask[b, 0, c * 128:(c + 1) * 128, k * 128:(k + 1) * 128]
            me[i] = _round_f32r(np.exp(np.minimum(blk, 80.0)).T)
        in_maps.append({
            "hsT": _round_f32r(np.ascontiguousarray(hsT_c)),
            "WqT": WqTs, "WkT": WkTs, "WvT": WvTs, "WoT": WoTs,
            "kTc": kTc_b[b], "Vc": Vc_b[b], "maskexp": me,
        })

    kw = {}
    if TRACE:
        kw = dict(trace=True, trace_cores=list(range(8)), **TRACE_KW)
    res = bass_utils.run_bass_kernel_spmd(nc, in_maps, core_ids=list(range(8)), **kw)
    LAST_EXEC_NS = res.exec_time_ns

    out = np.empty((B, S, D), np.float32)
    for ci in range(8):
        b, j = ci // 4, ci % 4
        arr = res.results[ci]["outT"]  # [16,128,QPC]
        for s, c in enumerate(CHUNK_ASSIGN[j]):
            blk = arr[:, :, s * 128:(s + 1) * 128]       # [jt, jj, qq]
            out[b, c * 128:(c + 1) * 128, :] = (
                blk.transpose(2, 0, 1).reshape(128, D))
    return out


# revision 4
# speedup vs baseline: 1.0833x; 1.0833x over previous
"""MultiHeadAttentionWithCache on 8 TRN2 NeuronCores (Bass/Tile).

Sharding: query rows. Cores 0-3 -> batch 0, cores 4-7 -> batch 1. Within a
batch, core j owns query chunks {j, j+4, 11-j, 15-j} (128 rows each) of the
16 chunks -- chosen so causal-mask work is balanced across cores.

Dataflow is fully "transposed" so the PE contracts on partition dims with no
on-device transposes:
  host supplies  hsT [D,512], weights pre-tiled per head/output tile,
                 kT_cache [H,HD,T] and v_cache tiled per batch.
  qT[h]   = (WqT_h).T @ hsT          [HD, 512]   (scale 1/sqrt(HD) folded in)
  k_newT[h], v_new = projections; AllGather over the 4 cores of the batch.
  scoresT = kT_tile.T @ qT           [keys, q]
  eT      = exp(scoresT) (* exp(mask) on partial tiles)   -- no max-subtract:
            |scores| <= ~10 for this problem, fp32 exp is safe.
  avT[h] += V_tile.T @ eT ;  sums += ones.T @ eT
  avT[h] /= sums  (partition_broadcast of reciprocal)
  outT    = WoT.T @ avT_all          [j, q]  -> host reassembles.

All matmuls run in float32r (TF32-like: 11-bit mantissa, full-rate PE).
Weight/cache tensors are pre-arranged on the host so every DMA moves
>=2KB-contiguous runs per partition (DMA descriptor efficiency).
"""
import math

import numpy as np

B, S, D, H, HD, CACHED = 2, 2048, 2048, 16, 128, 2048
T = CACHED + S
NKT = T // 128          # 32 key tiles
NCH = S // 128          # 16 query chunks per batch
QPC = 512               # query rows per core
NQT = QPC // 128        # 4 chunk slots per core

CHUNK_ASSIGN = [sorted([j, j + 4, 11 - j, 15 - j]) for j in range(4)]
_OWNER = {}
for _j, _chs in enumerate(CHUNK_ASSIGN):
    for _s, _c in enumerate(_chs):
        _OWNER[_c] = (_j, _s)

_PROG_CACHE = {}
TRACE = False
TRACE_KW = {}
LAST_EXEC_NS = None


def _round_f32r(x):
    u = np.ascontiguousarray(x, dtype=np.float32).view(np.uint32).astype(np.uint64)
    lsb = (u >> 12) & 1
    u2 = ((u + 0x7FF + lsb) & np.uint64(0xFFFFF000)).astype(np.uint32)
    return u2.view(np.float32)


def _classify_mask(attention_mask):
    """Block classification from the actual mask data.

    Returns (first, attended, partial_list):
      first[k]    : min slot whose moving range must include key tile k
      attended[k] : key tile computed at all
      partial_list: ordered [(s, k)] blocks needing an exp(mask) multiply
    Merged across all cores and batches (single SPMD program).
    """
    m = np.asarray(attention_mask)  # [B,1,S,T]
    blocks = m.reshape(B, NCH, 128, NKT, 128)
    bmax = blocks.max(axis=(2, 4))  # [B, NCH, NKT]
    bmin = blocks.min(axis=(2, 4))
    full = (np.abs(bmax) < 1e-30) & (np.abs(bmin) < 1e-30)  # mask == 0 block
    zero = bmax <= -200.0                                   # exp -> 0 block

    att_sk = np.zeros((NQT, NKT), dtype=bool)   # any core/batch attends
    full_sk = np.ones((NQT, NKT), dtype=bool)   # all cores/batches full
    for j in range(4):
        for s in range(NQT):
            c = CHUNK_ASSIGN[j][s]
            att_sk[s] |= (~zero[:, c, :]).any(axis=0)
            full_sk[s] &= full[:, c, :].all(axis=0)

    attended = att_sk.any(axis=0)
    attended[0] = True  # key tile 0 always computed full-width (PSUM start)
    first = np.full(NKT, 0, dtype=int)
    for k in range(NKT):
        if not attended[k]:
            continue
        s_att = np.where(att_sk[:, k])[0]
        first[k] = s_att.min() if len(s_att) else 0
        first[k] = min(first[k], 2)  # keep moving N >= 256 (f32r full rate)
    first[0] = 0

    partial_list = []
    for k in range(NKT):
        if not attended[k]:
            continue
        for s in range(first[k], NQT):
            if not full_sk[s, k]:
                partial_list.append((s, k))
    return first, attended, partial_list


def _build_program(first, attended, partial_list):
    from concourse import bacc
    import concourse.mybir as mybir
    import concourse.tile as tile

    f32 = mybir.dt.float32
    f32r = mybir.dt.float32r
    nblk = max(len(partial_list), 1)
    blk_idx = {sk: i for i, sk in enumerate(partial_list)}

    nc = bacc.Bacc("TRN2", num_devices=8)
    hsT = nc.dram_tensor("hsT", [D, QPC], f32r, kind="ExternalInput")
    # weights pre-tiled on host:
    Wq_r = nc.dram_tensor("Wq_r", [H, 128, 16, 128], f32r, kind="ExternalInput")
    Wk_r = nc.dram_tensor("Wk_r", [H, 128, 16, 128], f32r, kind="ExternalInput")
    Wv_r = nc.dram_tensor("Wv_r", [4, 16, 128, 512], f32r, kind="ExternalInput")
    Wo_r = nc.dram_tensor("Wo_r", [16, 128, 16, 128], f32r, kind="ExternalInput")
    kTc = nc.dram_tensor("kTc", [H, HD, CACHED], f32r, kind="ExternalInput")
    Vc = nc.dram_tensor("Vc", [H, 4, 128, 4, 128], f32r, kind="ExternalInput")
    maskexp = nc.dram_tensor("maskexp", [nblk, 128, 128], f32r, kind="ExternalInput")
    outT = nc.dram_tensor("outT", [16, 128, QPC], f32, kind="ExternalOutput")

    KSZ = H * HD * NQT * 128      # k region: [h, p, s, q]
    VSZ = H * 128 * NQT * 128     # v region: [h, p, s, d]
    with tile.TileContext(nc) as tc:
        with tc.tile_pool(name="sb", bufs=1) as sb, \
             tc.tile_pool(name="ps", bufs=1, space="PSUM") as ps, \
             tc.tile_pool(name="dram", bufs=1, space="DRAM") as dram:
            ag_in = dram.tile([KSZ + VSZ], f32r, name="ag_in")
            ag_out = dram.tile([4, KSZ + VSZ], f32r, name="ag_out")

            ones_f = sb.tile([128, 1], f32, name="ones_f")
            nc.gpsimd.memset(ones_f[:], 1.0)
            ones = sb.tile([128, 1], f32r, name="ones")
            nc.vector.tensor_copy(ones[:], ones_f[:])

            hsT_t = sb.tile([128, 16, QPC], f32r, name="hsT_t")
            nc.sync.dma_start(hsT_t[:], hsT.rearrange("(it p) q -> p it q", p=128))

            mb = sb.tile([128, nblk, 128], f32r, name="mb")
            nc.sync.dma_start(mb[:], maskexp.rearrange("n p q -> p n q"))

            qT_all = sb.tile([128, H, QPC], f32r, name="qT_all")
            avT_all = sb.tile([128, H, QPC], f32r, name="avT_all")

            ag_in_k = ag_in[:KSZ].rearrange("(h p s q) -> h p s q", h=H, p=HD, s=NQT)
            ag_in_v = ag_in[KSZ:].rearrange("(h p s d) -> h p s d", h=H, p=128, s=NQT)
            ag_out_k = ag_out[:, :KSZ].rearrange(
                "r (h p s q) -> r h p s q", h=H, p=HD, s=NQT)
            ag_out_v = ag_out[:, KSZ:].rearrange(
                "r (h p s d) -> r h p s d", h=H, p=128, s=NQT)

            # ---- K projection: k_newT[h] = WkT_h.T @ hsT  -> ag_in ----
            for h in range(H):
                wk = sb.tile([128, 16, 128], f32r, tag="wbig", bufs=2, name=f"wk{h}")
                nc.sync.dma_start(wk[:], Wk_r[h])
                pk = ps.tile([128, QPC], f32, tag="A", bufs=2, name=f"pk{h}")
                for it in range(16):
                    nc.tensor.matmul(pk[:], wk[:, it, :], hsT_t[:, it, :],
                                     start=(it == 0), stop=(it == 15))
                kst = sb.tile([128, QPC], f32r, tag="kst", bufs=2, name=f"kst{h}")
                nc.vector.tensor_copy(kst[:], pk[:])
                nc.sync.dma_start(ag_in_k[h], kst[:])

            # ---- V projection ----
            for cg in range(4):
                pvs = [ps.tile([128, 512], f32, tag=tg, bufs=2, name=f"pv{cg}_{s}")
                       for s, tg in enumerate(("A", "B", "C", "D"))]
                for it in range(16):
                    wv = sb.tile([128, 512], f32r, tag="wv", bufs=3,
                                 name=f"wv{cg}_{it}")
                    nc.sync.dma_start(wv[:], Wv_r[cg, it])
                    for s in range(4):
                        nc.tensor.matmul(
                            pvs[s][:], hsT_t[:, it, s * 128:(s + 1) * 128], wv[:],
                            start=(it == 0), stop=(it == 15))
                for s in range(4):
                    vst = sb.tile([128, 512], f32r, tag="vst", bufs=2,
                                  name=f"vst{cg}_{s}")
                    nc.vector.tensor_copy(vst[:], pvs[s][:])
                    for i in range(4):
                        h = cg * 4 + i
                        nc.sync.dma_start(ag_in_v[h, :, s, :],
                                          vst[:, i * 128:(i + 1) * 128])

            # ---- AllGather new K/V within each batch group ----
            nc.gpsimd.collective_compute(
                "AllGather", mybir.AluOpType.bypass,
                replica_groups=[[0, 1, 2, 3], [4, 5, 6, 7]],
                ins=[ag_in.opt()], outs=[ag_out.opt()],
            )

            # ---- Q projection (scale folded into Wq on host) ----
            for h in range(H):
                wq = sb.tile([128, 16, 128], f32r, tag="wbig", bufs=2, name=f"wq{h}")
                nc.sync.dma_start(wq[:], Wq_r[h])
                pq = ps.tile([128, QPC], f32, tag="B", bufs=2, name=f"pq{h}")
                for it in range(16):
                    nc.tensor.matmul(pq[:], wq[:, it, :], hsT_t[:, it, :],
                                     start=(it == 0), stop=(it == 15))
                nc.vector.tensor_copy(qT_all[:, h, :], pq[:])

            # ---- Attention per head ----
            att_keys = [k for k in range(NKT) if attended[k]]
            last_k = att_keys[-1]
            for h in range(H):
                kt_tiles = {}
                v_tiles = {}
                for g in range(4):  # cached groups of 4 key tiles
                    if not any(attended[k] for k in range(g * 4, g * 4 + 4)):
                        continue
                    kt = sb.tile([128, 512], f32r, tag="kt", bufs=4,
                                 name=f"kt{h}_{g}")
                    vt = sb.tile([128, 4, HD], f32r, tag="vt", bufs=4,
                                 name=f"vt{h}_{g}")
                    nc.sync.dma_start(kt[:], kTc[h, :, g * 512:(g + 1) * 512])
                    nc.sync.dma_start(vt[:], Vc[h, g])
                    kt_tiles[g] = kt
                    v_tiles[g] = vt
                if any(attended[k] for k in range(16, 32)):
                    ktn = sb.tile([128, 4, NQT, 128], f32r, tag="ktn", bufs=1,
                                  name=f"ktn{h}")
                    vtn = sb.tile([128, 4, NQT, 128], f32r, tag="vtn", bufs=1,
                                  name=f"vtn{h}")
                    for r in range(4):
                        nc.sync.dma_start(ktn[:, r], ag_out_k[r, h])
                        nc.sync.dma_start(vtn[:, r], ag_out_v[r, h])

                avp = ps.tile([128, QPC], f32, tag="C", bufs=2, name=f"av{h}")
                smp = ps.tile([1, QPC], f32, tag="D", bufs=2, name=f"sm{h}")
                for k in att_keys:
                    f = first[k]
                    n = QPC - f * 128
                    if k < 16:
                        g, i = k // 4, k % 4
                        k_st = kt_tiles[g][:, i * 128:(i + 1) * 128]
                        v_st = v_tiles[g][:, i, :]
                    else:
                        r, sl = _OWNER[k - 16]
                        k_st = ktn[:, r, sl, :]
                        v_st = vtn[:, r, sl, :]
                    sc = ps.tile([128, n], f32, tag="A", bufs=2, name=f"sc{h}_{k}")
                    nc.tensor.matmul(sc[:], k_st, qT_all[:, h, f * 128:],
                                     start=True, stop=True)
                    eT = sb.tile([128, n], f32r, tag="eT", bufs=4,
                                 name=f"eT{h}_{k}")
                    nc.scalar.activation(eT[:], sc[:],
                                         mybir.ActivationFunctionType.Exp)
                    for s in range(f, NQT):
                        bi = blk_idx.get((s, k))
                        if bi is not None:
                            off = (s - f) * 128
                            nc.vector.tensor_mul(
                                eT[:, off:off + 128], eT[:, off:off + 128],
                                mb[:, bi, :])
                    nc.tensor.matmul(avp[:, f * 128:], v_st, eT[:],
                                     start=(k == 0), stop=(k == last_k))
                    nc.tensor.matmul(smp[:, f * 128:], ones[:], eT[:],
                                     start=(k == 0), stop=(k == last_k))

                recip = sb.tile([1, QPC], f32, tag="recip", bufs=2, name=f"rc{h}")
                nc.vector.reciprocal(recip[:], smp[:])
                rep = sb.tile([128, QPC], f32, tag="rep", bufs=2, name=f"rep{h}")
                nc.gpsimd.partition_broadcast(rep[:], recip[:])
                nc.vector.tensor_mul(avT_all[:, h, :], avp[:], rep[:])

            # ---- Output projection ----
            for jt in range(16):
                wo = sb.tile([128, 16, 128], f32r, tag="wbig", bufs=2,
                             name=f"wo{jt}")
                nc.sync.dma_start(wo[:], Wo_r[jt])
                po = ps.tile([128, QPC], f32, tag="B", bufs=2, name=f"po{jt}")
                for ct in range(16):
                    nc.tensor.matmul(po[:], wo[:, ct, :], avT_all[:, ct, :],
                                     start=(ct == 0), stop=(ct == 15))
                ob = sb.tile([128, QPC], f32, tag="ob", bufs=2, name=f"ob{jt}")
                nc.vector.tensor_copy(ob[:], po[:])
                nc.sync.dma_start(outT[jt], ob[:])

    nc.compile()
    return nc


def _prep_weights(Wq, Wk, Wv, Wo):
    WqT = np.asarray(Wq, np.float32).T * (HD ** -0.5)
    WkT = np.asarray(Wk, np.float32).T
    WvT = np.asarray(Wv, np.float32).T
    WoT = np.asarray(Wo, np.float32).T
    # [in,out] -> [h, p, it, c]
    wq_r = _round_f32r(WqT.reshape(16, 128, 16, 128).transpose(2, 1, 0, 3))
    wk_r = _round_f32r(WkT.reshape(16, 128, 16, 128).transpose(2, 1, 0, 3))
    # [in,out] -> [cg, it, p, c]
    wv_r = _round_f32r(WvT.reshape(16, 128, 4, 512).transpose(2, 0, 1, 3))
    # [c,j]   -> [jt, p, ct, j]
    wo_r = _round_f32r(WoT.reshape(16, 128, 16, 128).transpose(2, 1, 0, 3))
    return wq_r, wk_r, wv_r, wo_r


def kernel(hidden_states, Wq, Wk, Wv, Wo, k_cache, v_cache, attention_mask):
    global LAST_EXEC_NS
    from concourse import bass_utils

    hs = np.asarray(hidden_states, dtype=np.float32)
    mask = np.asarray(attention_mask, dtype=np.float32)

    first, attended, partial_list = _classify_mask(mask)
    sig = (tuple(first), tuple(bool(a) for a in attended), tuple(partial_list))
    if sig not in _PROG_CACHE:
        _PROG_CACHE[sig] = _build_program(first, attended, partial_list)
    nc = _PROG_CACHE[sig]

    wq_r, wk_r, wv_r, wo_r = _prep_weights(Wq, Wk, Wv, Wo)
    kTc_b = [_round_f32r(np.ascontiguousarray(
        np.asarray(k_cache, np.float32)[b].transpose(0, 2, 1))) for b in range(B)]
    # v_cache [H,2048,128] -> [h, g, p, n, d]
    Vc_b = [_round_f32r(np.asarray(v_cache, np.float32)[b]
                        .reshape(H, 4, 4, 128, 128).transpose(0, 1, 3, 2, 4))
            for b in range(B)]

    nblk = max(len(partial_list), 1)
    in_maps = []
    for ci in range(8):
        b, j = ci // 4, ci % 4
        chunks = CHUNK_ASSIGN[j]
        hsT_c = np.concatenate(
            [hs[b, c * 128:(c + 1) * 128, :] for c in chunks], axis=0).T
        me = np.zeros((nblk, 128, 128), np.float32)
        for i, (s, k) in enumerate(partial_list):
            c = chunks[s]
            blk = mask[b, 0, c * 128:(c + 1) * 128, k * 128:(k + 1) * 128]
            me[i] = _round_f32r(np.exp(np.minimum(blk, 80.0)).T)
        in_maps.append({
            "hsT": _round_f32r(np.ascontiguousarray(hsT_c)),
            "Wq_r": wq_r, "Wk_r": wk_r, "Wv_r": wv_r, "Wo_r": wo_r,
            "kTc": kTc_b[b], "Vc": Vc_b[b], "maskexp": me,
        })

    kw = {}
    if TRACE:
        kw = dict(trace=True, trace_cores=list(range(8)), **TRACE_KW)
    res = bass_utils.run_bass_kernel_spmd(nc, in_maps, core_ids=list(range(8)), **kw)
    LAST_EXEC_NS = res.exec_time_ns

    out = np.empty((B, S, D), np.float32)
    for ci in range(8):
        b, j = ci // 4, ci % 4
        arr = res.results[ci]["outT"]  # [16,128,QPC]
        for s, c in enumerate(CHUNK_ASSIGN[j]):
            blk = arr[:, :, s * 128:(s + 1) * 128]       # [jt, jj, qq]
            out[b, c * 128:(c + 1) * 128, :] = (
                blk.transpose(2, 0, 1).reshape(128, D))
    return out
